# revision 13
# baseline (speedup 1.0000x reference)
"""BiGRU encoder kernel for 8 Trainium2 NeuronCores.

Strategy (v3, fp8 DoubleRow + quarter-tile streams):
  - Masked GRU over FIXED position ranges: forward runs positions (8-n)..7
    ascending, backward (6+n)..7 descending; a sample of length l starts at
    step n-l with h=0 (prefix memset) and a +BIG z-gate mask keeps
    over-included samples at exactly h=0 until their true start.
  - Sort samples by window_len, deal round-robin to 8 cores; per core FOUR
    batch tiles of 256 sorted samples -> 8 independent (tile, direction)
    streams.  Streams are end-staggered so every phase of the kernel has
    several streams in flight (gate latency of one hides under matmuls of
    others).  Step widths are EXACT per-step active counts (max over cores,
    rounded to 16 for alignment); the <=16 over-included samples are fixed
    by a narrow z-mask add into PSUM.
  - Matmuls: fp8e4 DoubleRow (K=256 per instruction, measured 2x throughput)
    for ALL hidden projections and for input projections except the last
    N_HI=3 steps of each stream, which run in bf16 for accuracy.  Weights are
    pre-scaled by 512 (exact power of 2) so unscaled fp8 x/h stay in e4m3's
    normal range; activations rescale with scale=1/512.
  - h is carried in bf16 (fp8 carry compounds error); the fp8 copy for the
    next step's matmul is written by vector (critical path), the bf16 carry
    by gpsimd (off critical path).
  - Hidden projections run at the PREVIOUS step's width; the n-gate
    pre-activation prefix (newly started samples) is r*bhh_n via a scalar
    ACT Copy with per-partition scale.
  - Output is written feature-major [H, Bc]; the host transposes (free).
"""

import os
from contextlib import ExitStack

import numpy as np
import ml_dtypes

import concourse.bacc as bacc
import concourse.tile as tile
from concourse import mybir
from concourse.bass_utils import run_bass_kernel_spmd

NCORES = 8
B, T, D, H = 8192, 15, 512, 512
G = 3 * H
BIG = 40.0
TS = 256             # samples per batch tile
NTILES = 4
BC = TS * NTILES     # samples per core
N_HI = int(os.environ.get("GRU_NHI", "3"))  # last-k steps with bf16 input proj
F32 = mybir.dt.float32
BF16 = mybir.dt.bfloat16
F8 = mybir.dt.float8e4
DR = mybir.MatmulPerfMode.DoubleRow

ACT = mybir.ActivationFunctionType
ALU = mybir.AluOpType

NP_BF = ml_dtypes.bfloat16
NP_F8 = ml_dtypes.float8_e4m3

_PROGRAM_CACHE = {}
LAST_RESULT = None


def _build_program(sched):
    """sched[t][d] = tuple of (w, w_prev_hidden, mw, hi) per step."""
    nc = bacc.Bacc("TRN2", target_bir_lowering=False, debug=False,
                   num_devices=NCORES)

    x8_d = nc.dram_tensor("x8", [T, D, BC], F8, kind="ExternalInput")
    xb_d = nc.dram_tensor("xb", [T, D, BC], BF16, kind="ExternalInput")
    w8f_d = nc.dram_tensor("w8f", [D + H, G], F8, kind="ExternalInput")
    w8b_d = nc.dram_tensor("w8b", [D + H, G], F8, kind="ExternalInput")
    wbf_d = nc.dram_tensor("wbf", [D, G], BF16, kind="ExternalInput")
    wbb_d = nc.dram_tensor("wbb", [D, G], BF16, kind="ExternalInput")
    w1_d = nc.dram_tensor("w1", [2 * H, H], BF16, kind="ExternalInput")
    w2_d = nc.dram_tensor("w2", [H, H], BF16, kind="ExternalInput")
    bias_d = nc.dram_tensor("bias", [40, 128], F32, kind="ExternalInput")
    mf_d = nc.dram_tensor("maskzf", [8, BC], BF16, kind="ExternalInput")
    mb_d = nc.dram_tensor("maskzb", [8, BC], BF16, kind="ExternalInput")
    y_d = nc.dram_tensor("y", [H, BC], F32, kind="ExternalOutput")

    NS = 2 * NTILES  # streams
    with tile.TileContext(nc) as tc, ExitStack() as ctx:
        const = ctx.enter_context(tc.tile_pool(name="const", bufs=1))
        x8pool = ctx.enter_context(tc.tile_pool(name="x8", bufs=6))
        xbpool = ctx.enter_context(tc.tile_pool(name="xb", bufs=6))
        hbf = [ctx.enter_context(tc.tile_pool(name=f"hb{s}", bufs=2))
               for s in range(NS)]
        hf8 = [ctx.enter_context(tc.tile_pool(name=f"h8{s}", bufs=2))
               for s in range(NS)]
        hfin = ctx.enter_context(tc.tile_pool(name="hfin", bufs=NS))
        gpool = ctx.enter_context(tc.tile_pool(name="g", bufs=24))
        mpool = ctx.enter_context(tc.tile_pool(name="m", bufs=2))
        opool = ctx.enter_context(tc.tile_pool(name="o", bufs=2))
        rzps = ctx.enter_context(tc.tile_pool(name="rz", bufs=5, space="PSUM"))
        xpps = ctx.enter_context(tc.tile_pool(name="xp", bufs=2, space="PSUM"))
        ghps = ctx.enter_context(tc.tile_pool(name="gh", bufs=1, space="PSUM"))

        def wtile(dram, kchunks, cols, dt, name):
            t_ = const.tile([128, kchunks, cols], dt, name=name)
            return t_, dram.rearrange("(c k) g -> k c g", k=128)

        w8f_t, w8f_s = wtile(w8f_d, 8, G, F8, "w8f")
        w8b_t, w8b_s = wtile(w8b_d, 8, G, F8, "w8b")
        wbf_t, wbf_s = wtile(wbf_d, 4, G, BF16, "wbf")
        wbb_t, wbb_s = wtile(wbb_d, 4, G, BF16, "wbb")
        w1, w1_s = wtile(w1_d, 8, H, BF16, "w1")
        w2, w2_s = wtile(w2_d, 4, H, BF16, "w2")
        w8 = [w8f_t, w8b_t]
        wbf = [wbf_t, wbb_t]
        bt = const.tile([128, 40], F32)
        nc.gpsimd.dma_start(bt[:], bias_d.rearrange("n p -> p n"))
        # DMA order = order of first use.  The scalar queue must stay clear
        # early (ACT table load + first gates run there); weights go on
        # sync/gpsimd in need-order, w1/w2 go on scalar after step 2.
        for c in range(4):  # fp8 input chunks first (step 0 needs them)
            nc.sync.dma_start(w8f_t[:, c, :], w8f_s[:, c, :])
            nc.gpsimd.dma_start(w8b_t[:, c, :], w8b_s[:, c, :])
        for c in range(4):  # bf16 input weights (short streams hit hi early)
            nc.gpsimd.dma_start(wbf_t[:, c, :], wbf_s[:, c, :])
            nc.gpsimd.dma_start(wbb_t[:, c, :], wbb_s[:, c, :])
        mask_d = [mf_d, mb_d]

        def load_late_weights_a():  # fp8 hidden chunks (needed from step 1)
            for c in range(4, 8):
                nc.sync.dma_start(w8f_t[:, c, :], w8f_s[:, c, :])
                nc.sync.dma_start(w8b_t[:, c, :], w8b_s[:, c, :])

        def load_late_weights_b():  # MLP weights (needed after short streams)
            for c in range(8):
                nc.scalar.dma_start(w1[:, c, :], w1_s[:, c, :])
            for c in range(4):
                nc.scalar.dma_start(w2[:, c, :], w2_s[:, c, :])

        state = {}

        def emit_step(t, d, j, steps):
            w, wh, mw, hi = steps[j]
            n = len(steps)
            first = j == 0
            last = j == n - 1
            so = TS - w
            soh = TS - wh if not first else None  # hidden-proj suffix offset
            a0 = t * TS + so
            bb = 16 * d
            pos = (8 - n + j) if d == 0 else (6 + n - j)
            skey = 2 * t + d

            if hi:
                xt = xbpool.tile([128, 4, TS], BF16, tag="xb", name="xt")
                nc.sync.dma_start(
                    xt[:, :, so:],
                    xb_d[pos].rearrange("(c k) s -> k c s", k=128)[:, :, a0:a0 + w])
            else:
                xt = x8pool.tile([128, 4, TS], F8, tag="x8", name="xt")
                nc.sync.dma_start(
                    xt[:, :, so:],
                    x8_d[pos].rearrange("(c k) s -> k c s", k=128)[:, :, a0:a0 + w])
            mt = None
            if mw:
                mt = mpool.tile([128, 64], BF16, tag="m", name="mt")
                nc.gpsimd.dma_start(
                    mt[:, :mw],
                    mask_d[d][8 - (n - j), a0:a0 + mw].partition_broadcast(128))

            if first:
                h_prev = h8_prev = None
            else:
                h_prev, h8_prev, _ = state[skey]
            hb_next = (hfin if last else hbf[skey]).tile(
                [128, 4, TS], BF16, tag="hf" if last else f"h{skey}", name="hb")
            h8_next = None
            if not last:
                h8_next = hf8[skey].tile([128, 4, TS], F8, tag=f"g8{skey}",
                                         name="h8")
                nw = steps[j + 1][0]
                if TS - nw < so:  # zero newly exposed prefix for next step's dd
                    nc.gpsimd.memset(hb_next[:, :, TS - nw:so].bitcast(F32), 0.0)

            rps, zps, xpns, ghns = [], [], [], []
            for i in range(4):
                c0 = i * 128
                r_ps = rzps.tile([128, w], F32, tag="rz", name=f"rps{i}")
                z_ps = rzps.tile([128, w], F32, tag="rz", name=f"zps{i}")
                xpn = xpps.tile([128, w], F32, tag="xp", name=f"xpn{i}")
                rps.append(r_ps); zps.append(z_ps); xpns.append(xpn)
                if hi:
                    ww = wbf[d]
                    for k in range(4):
                        st = k == 0
                        lastk = k == 3 and first
                        nc.tensor.matmul(r_ps[:], ww[:, k, c0:c0 + 128],
                                         xt[:, k, so:], start=st, stop=lastk)
                        nc.tensor.matmul(z_ps[:], ww[:, k, H + c0:H + c0 + 128],
                                         xt[:, k, so:], start=st, stop=lastk)
                        nc.tensor.matmul(xpn[:], ww[:, k, 2 * H + c0:2 * H + c0 + 128],
                                         xt[:, k, so:], start=st, stop=k == 3)
                else:
                    ww = w8[d]
                    for p in range(2):
                        st = p == 0
                        lastk = p == 1 and first
                        ksl = slice(2 * p, 2 * p + 2)
                        nc.tensor.matmul(r_ps[:], ww[:, ksl, c0:c0 + 128],
                                         xt[:, ksl, so:], start=st, stop=lastk,
                                         perf_mode=DR)
                        nc.tensor.matmul(z_ps[:], ww[:, ksl, H + c0:H + c0 + 128],
                                         xt[:, ksl, so:], start=st, stop=lastk,
                                         perf_mode=DR)
                        nc.tensor.matmul(xpn[:], ww[:, ksl, 2 * H + c0:2 * H + c0 + 128],
                                         xt[:, ksl, so:], start=st, stop=p == 1,
                                         perf_mode=DR)
                if not first:
                    wwh = w8[d]
                    ghn = ghps.tile([128, wh], F32, tag="gh", name=f"ghn{i}")
                    ghns.append(ghn)
                    for p in range(2):
                        ksl = slice(4 + 2 * p, 4 + 2 * p + 2)
                        nc.tensor.matmul(rps[i][:, soh - so:],
                                         wwh[:, ksl, c0:c0 + 128],
                                         h8_prev[:, 2 * p:2 * p + 2, soh:],
                                         start=False, stop=p == 1, perf_mode=DR)
                        nc.tensor.matmul(zps[i][:, soh - so:],
                                         wwh[:, ksl, H + c0:H + c0 + 128],
                                         h8_prev[:, 2 * p:2 * p + 2, soh:],
                                         start=False, stop=p == 1, perf_mode=DR)
                        nc.tensor.matmul(ghn[:],
                                         wwh[:, ksl, 2 * H + c0:2 * H + c0 + 128],
                                         h8_prev[:, 2 * p:2 * p + 2, soh:],
                                         start=p == 0, stop=p == 1, perf_mode=DR)

            for i in range(4):
                if mw:
                    nc.vector.tensor_add(zps[i][:, :mw], zps[i][:, :mw],
                                         mt[:, :mw])
                r = gpool.tile([128, w], BF16, tag="g", name="r")
                nc.scalar.activation(r[:], rps[i][:], ACT.Sigmoid,
                                     bias=bt[:, bb + i:bb + i + 1],
                                     scale=1.0 / 512)
                z = gpool.tile([128, w], BF16, tag="g", name="z")
                nc.scalar.activation(z[:], zps[i][:], ACT.Sigmoid,
                                     bias=bt[:, bb + 4 + i:bb + 5 + i],
                                     scale=1.0 / 512)
                tt = gpool.tile([128, w], BF16, tag="g", name="tt")
                if first:
                    # tt = r * bhh_n  via ACT Copy with per-partition scale
                    nc.scalar.activation(tt[:], r[:], ACT.Copy, bias=0.0,
                                         scale=bt[:, bb + 8 + i:bb + 9 + i])
                else:
                    dd = soh - so
                    if dd:
                        nc.scalar.activation(tt[:, :dd], r[:, :dd], ACT.Copy,
                                             bias=0.0,
                                             scale=bt[:, bb + 8 + i:bb + 9 + i])
                    nc.vector.scalar_tensor_tensor(
                        tt[:, dd:], ghns[i][:], bt[:, bb + 8 + i:bb + 9 + i],
                        r[:, dd:], op0=ALU.add, op1=ALU.mult)
                ss = gpool.tile([128, w], BF16, tag="g", name="ss")
                nc.vector.tensor_add(ss[:], tt[:], xpns[i][:])
                nn = gpool.tile([128, w], BF16, tag="g", name="n")
                nc.scalar.activation(nn[:], ss[:], ACT.Tanh,
                                     bias=bt[:, bb + 12 + i:bb + 13 + i],
                                     scale=1.0 / 512)
                # critical chain (feeds next step's hidden matmuls via h8)
                # stays on vector; the bf16 carry copy goes to gpsimd.
                ho = hb_next[:, i, so:]
                if first:
                    e = gpool.tile([128, w], BF16, tag="g", name="e")
                    nc.vector.tensor_mul(e[:], z[:], nn[:])
                    if h8_next is not None:
                        nc.vector.tensor_sub(h8_next[:, i, so:], nn[:], e[:])
                        nc.gpsimd.tensor_sub(ho, nn[:], e[:])
                    else:
                        nc.vector.tensor_sub(ho, nn[:], e[:])
                else:
                    dd_t = gpool.tile([128, w], BF16, tag="g", name="dd")
                    nc.vector.tensor_sub(dd_t[:], h_prev[:, i, so:], nn[:])
                    e = gpool.tile([128, w], BF16, tag="g", name="e")
                    nc.vector.tensor_mul(e[:], z[:], dd_t[:])
                    if h8_next is not None:
                        nc.vector.tensor_add(h8_next[:, i, so:], nn[:], e[:])
                        nc.gpsimd.tensor_add(ho, nn[:], e[:])
                    else:
                        nc.vector.tensor_add(ho, nn[:], e[:])
            state[skey] = (hb_next, h8_next, w)
            return hb_next

        def emit_mlp(t, hf_t, hb_t):
            hid = []
            for i in range(4):
                ps = xpps.tile([128, TS], F32, tag="xp", name="mps")
                for k in range(8):
                    src = hf_t if k < 4 else hb_t
                    nc.tensor.matmul(ps[:], w1[:, k, i * 128:(i + 1) * 128],
                                     src[:, k % 4, :], start=k == 0, stop=k == 7)
                hr = gpool.tile([128, TS], BF16, tag="g", name="hr")
                nc.scalar.activation(hr[:], ps[:], ACT.Relu,
                                     bias=bt[:, 32 + i:33 + i])
                hid.append(hr)
            for i in range(4):
                ps = xpps.tile([128, TS], F32, tag="xp", name="ops")
                for k in range(4):
                    nc.tensor.matmul(ps[:], w2[:, k, i * 128:(i + 1) * 128],
                                     hid[k][:], start=k == 0, stop=k == 3)
                o32 = opool.tile([128, TS], F32, tag="o", name="o32")
                nc.vector.tensor_scalar_add(o32[:], ps[:], bt[:, 36 + i:37 + i])
                nc.sync.dma_start(y_d[i * 128:(i + 1) * 128, t * TS:(t + 1) * TS],
                                  o32[:])

        # End-staggered interleave: later (longer) quarters end later, so
        # every super-step has several streams in flight and the quarter
        # MLPs drain progressively instead of piling up at the end.
        starts = {}
        nmax = 0
        for t in range(NTILES):
            for d in range(2):
                n = len(sched[t][d])
                endoff = NTILES - 1 - t
                starts[(t, d)] = endoff  # provisional: start offset set below
                nmax = max(nmax, n + endoff)
        for t in range(NTILES):
            for d in range(2):
                n = len(sched[t][d])
                endoff = NTILES - 1 - t
                starts[(t, d)] = nmax - n - endoff

        hfs = {}
        mlp_done = set()
        for J in range(nmax):
            for t in range(NTILES):
                for d in range(2):
                    steps = sched[t][d]
                    j = J - starts[(t, d)]
                    if 0 <= j < len(steps):
                        h = emit_step(t, d, j, steps)
                        if j == len(steps) - 1:
                            hfs[(t, d)] = h
            if J == 0:
                load_late_weights_a()
            elif J == 2:
                load_late_weights_b()
            for t in range(NTILES):
                if t not in mlp_done and (t, 0) in hfs and (t, 1) in hfs:
                    emit_mlp(t, hfs[(t, 0)], hfs[(t, 1)])
                    mlp_done.add(t)

    nc.compile()
    return nc


def _mk_sched(lens_pc, t):
    """lens_pc: [BC, NCORES] per-core sorted lengths; tile t rows."""
    seg = lens_pc[t * TS:(t + 1) * TS]  # [TS, NCORES]
    n = int(seg.max())
    steps = []
    for j in range(n):
        need = n - j
        cnt = (seg >= need).sum(axis=0)
        w = min(TS, -(-int(cnt.max()) // 16) * 16)
        mw = int(w - int(cnt.min()))
        hi = j >= n - N_HI
        steps.append([w, 0, mw, hi])
    for j in range(1, n):
        steps[j][1] = steps[j - 1][0]  # hidden width = prev step width
    return tuple(tuple(s) for s in steps)


def kernel(padded_window, window_len, Wih_f, Whh_f, bih_f, bhh_f,
           Wih_b, Whh_b, bih_b, bhh_b, W1, b1, W2, b2):
    wl = np.asarray(window_len)
    lf = (wl - 1) // 2 + 1
    lb = wl // 2 + 1
    order = np.argsort(wl, kind="stable")

    lf_pc = lf[order].reshape(-1, NCORES)
    lb_pc = lb[order].reshape(-1, NCORES)

    sched = tuple((_mk_sched(lf_pc, t), _mk_sched(lb_pc, t))
                  for t in range(NTILES))

    if sched not in _PROGRAM_CACHE:
        _PROGRAM_CACHE[sched] = _build_program(sched)
    nc = _PROGRAM_CACHE[sched]

    f32 = np.float32
    wf_full = np.concatenate([Wih_f.T, Whh_f.T], 0).astype(f32) * 512.0
    wb_full = np.concatenate([Wih_b.T, Whh_b.T], 0).astype(f32) * 512.0
    w8f = np.clip(wf_full, -240, 240).astype(NP_F8)
    w8b = np.clip(wb_full, -240, 240).astype(NP_F8)
    wbf = wf_full[:D].astype(NP_BF)
    wbb = wb_full[:D].astype(NP_BF)
    w1 = np.ascontiguousarray(W1.T, dtype=f32).astype(NP_BF)
    w2 = np.ascontiguousarray(W2.T, dtype=f32).astype(NP_BF)

    def chunks(v):
        return np.asarray(v, f32).reshape(4, 128)

    bias = np.concatenate([
        chunks((bih_f + bhh_f)[:H]), chunks((bih_f + bhh_f)[H:2 * H]),
        chunks(bhh_f[2 * H:] * 512.0), chunks(bih_f[2 * H:]),
        chunks((bih_b + bhh_b)[:H]), chunks((bih_b + bhh_b)[H:2 * H]),
        chunks(bhh_b[2 * H:] * 512.0), chunks(bih_b[2 * H:]),
        chunks(b1), chunks(b2),
    ], 0)  # [40, 128]

    pw = np.asarray(padded_window, f32)
    in_maps = []
    p8 = np.arange(8)
    for c in range(NCORES):
        idx = order[c::NCORES]
        xT = np.ascontiguousarray(pw[idx].transpose(1, 2, 0))  # [15, 512, BC]
        mzf = (512.0 * BIG * (p8[:, None] < (8 - lf[idx])[None, :])).astype(NP_BF)
        mzb = (512.0 * BIG * (p8[:, None] < (8 - lb[idx])[None, :])).astype(NP_BF)
        in_maps.append({
            "x8": np.clip(xT, -240, 240).astype(NP_F8),
            "xb": xT.astype(NP_BF),
            "w8f": w8f, "w8b": w8b, "wbf": wbf, "wbb": wbb,
            "w1": w1, "w2": w2, "bias": bias,
            "maskzf": mzf, "maskzb": mzb,
        })

    trace = bool(os.environ.get("GRU_TRACE"))
    kw = {}
    if os.environ.get("GRU_TMPDIR"):
        kw["tmpdir"] = os.environ["GRU_TMPDIR"]
    res = run_bass_kernel_spmd(nc, in_maps, core_ids=list(range(NCORES)),
                               trace=trace, **kw)
    global LAST_RESULT
    LAST_RESULT = res
    out = np.empty((B, H), f32)
    for c in range(NCORES):
        out[order[c::NCORES]] = res.results[c]["y"].T
    return out


# revision 18
# speedup vs baseline: 1.0081x; 1.0081x over previous
"""BiGRU encoder kernel for 8 Trainium2 NeuronCores.

Strategy (v3, fp8 DoubleRow + quarter-tile streams):
  - Masked GRU over FIXED position ranges: forward runs positions (8-n)..7
    ascending, backward (6+n)..7 descending; a sample of length l starts at
    step n-l with h=0 (prefix memset) and a +BIG z-gate mask keeps
    over-included samples at exactly h=0 until their true start.
  - Sort samples by window_len, deal round-robin to 8 cores; per core FOUR
    batch tiles of 256 sorted samples -> 8 independent (tile, direction)
    streams.  Streams are end-staggered so every phase of the kernel has
    several streams in flight (gate latency of one hides under matmuls of
    others).  Step widths are EXACT per-step active counts (max over cores,
    rounded to 16 for alignment); the <=16 over-included samples are fixed
    by a narrow z-mask add into PSUM.
  - Matmuls: fp8e4 DoubleRow (K=256 per instruction, measured 2x throughput)
    for ALL hidden projections and for input projections except the last
    N_HI=3 steps of each stream, which run in bf16 for accuracy.  Weights are
    pre-scaled by 512 (exact power of 2) so unscaled fp8 x/h stay in e4m3's
    normal range; activations rescale with scale=1/512.
  - h is carried in bf16 (fp8 carry compounds error); the fp8 copy for the
    next step's matmul is written by vector (critical path), the bf16 carry
    by gpsimd (off critical path).
  - Hidden projections run at the PREVIOUS step's width; the n-gate
    pre-activation prefix (newly started samples) is r*bhh_n via a scalar
    ACT Copy with per-partition scale.
  - Output is written feature-major [H, Bc]; the host transposes (free).
"""

import os
from contextlib import ExitStack

import numpy as np
import ml_dtypes

import concourse.bacc as bacc
import concourse.tile as tile
from concourse import mybir
from concourse.bass_utils import run_bass_kernel_spmd

NCORES = 8
B, T, D, H = 8192, 15, 512, 512
G = 3 * H
BIG = 40.0
TS = 512             # samples per batch tile
NTILES = 2
BC = TS * NTILES     # samples per core
N_HI_N = int(os.environ.get("GRU_NHI_N", "4"))   # n-gate bf16 on last-k steps
N_HI_RZ = int(os.environ.get("GRU_NHI_RZ", "1"))  # r/z bf16 on last-k steps
F32 = mybir.dt.float32
BF16 = mybir.dt.bfloat16
F8 = mybir.dt.float8e4
DR = mybir.MatmulPerfMode.DoubleRow

ACT = mybir.ActivationFunctionType
ALU = mybir.AluOpType

NP_BF = ml_dtypes.bfloat16
NP_F8 = ml_dtypes.float8_e4m3

_PROGRAM_CACHE = {}
LAST_RESULT = None


def _build_program(sched):
    """sched[t][d] = tuple of (w, w_prev_hidden, mw, hi) per step."""
    nc = bacc.Bacc("TRN2", target_bir_lowering=False, debug=False,
                   num_devices=NCORES)

    x8_d = nc.dram_tensor("x8", [T, D, BC], F8, kind="ExternalInput")
    xb_d = nc.dram_tensor("xb", [T, D, BC], BF16, kind="ExternalInput")
    w8f_d = nc.dram_tensor("w8f", [D + H, G], F8, kind="ExternalInput")
    w8b_d = nc.dram_tensor("w8b", [D + H, G], F8, kind="ExternalInput")
    wbf_d = nc.dram_tensor("wbf", [D, G], BF16, kind="ExternalInput")
    wbb_d = nc.dram_tensor("wbb", [D, G], BF16, kind="ExternalInput")
    w1_d = nc.dram_tensor("w1", [2 * H, H], BF16, kind="ExternalInput")
    w2_d = nc.dram_tensor("w2", [H, H], BF16, kind="ExternalInput")
    bias_d = nc.dram_tensor("bias", [40, 128], F32, kind="ExternalInput")
    mf_d = nc.dram_tensor("maskzf", [8, BC], BF16, kind="ExternalInput")
    mb_d = nc.dram_tensor("maskzb", [8, BC], BF16, kind="ExternalInput")
    y_d = nc.dram_tensor("y", [H, BC], F32, kind="ExternalOutput")

    NS = 2 * NTILES  # streams
    with tile.TileContext(nc) as tc, ExitStack() as ctx:
        const = ctx.enter_context(tc.tile_pool(name="const", bufs=1))
        x8pool = ctx.enter_context(tc.tile_pool(name="x8", bufs=6))
        xbpool = ctx.enter_context(tc.tile_pool(name="xb", bufs=6))
        hbf = [ctx.enter_context(tc.tile_pool(name=f"hb{s}", bufs=2))
               for s in range(NS)]
        hf8 = [ctx.enter_context(tc.tile_pool(name=f"h8{s}", bufs=2))
               for s in range(NS)]
        hfin = ctx.enter_context(tc.tile_pool(name="hfin", bufs=NS))
        gpool = ctx.enter_context(tc.tile_pool(name="g", bufs=28))
        mpool = ctx.enter_context(tc.tile_pool(name="m", bufs=2))
        opool = ctx.enter_context(tc.tile_pool(name="o", bufs=2))
        rzps = ctx.enter_context(tc.tile_pool(name="rz", bufs=5, space="PSUM"))
        xpps = ctx.enter_context(tc.tile_pool(name="xp", bufs=2, space="PSUM"))
        ghps = ctx.enter_context(tc.tile_pool(name="gh", bufs=1, space="PSUM"))

        def wtile(dram, kchunks, cols, dt, name):
            t_ = const.tile([128, kchunks, cols], dt, name=name)
            return t_, dram.rearrange("(c k) g -> k c g", k=128)

        w8f_t, w8f_s = wtile(w8f_d, 8, G, F8, "w8f")
        w8b_t, w8b_s = wtile(w8b_d, 8, G, F8, "w8b")
        wbf_t, wbf_s = wtile(wbf_d, 4, G, BF16, "wbf")
        wbb_t, wbb_s = wtile(wbb_d, 4, G, BF16, "wbb")
        w1, w1_s = wtile(w1_d, 8, H, BF16, "w1")
        w2, w2_s = wtile(w2_d, 4, H, BF16, "w2")
        w8 = [w8f_t, w8b_t]
        wbf = [wbf_t, wbb_t]
        bt = const.tile([128, 40], F32)
        nc.gpsimd.dma_start(bt[:], bias_d.rearrange("n p -> p n"))
        # DMA order = order of first use.  The scalar queue must stay clear
        # early (ACT table load + first gates run there); weights go on
        # sync/gpsimd in need-order, w1/w2 go on scalar after step 2.
        for c in range(4):  # fp8 input chunks first (step 0 needs them)
            nc.sync.dma_start(w8f_t[:, c, :], w8f_s[:, c, :])
            nc.gpsimd.dma_start(w8b_t[:, c, :], w8b_s[:, c, :])
        for c in range(4):  # bf16 input weights (short streams hit hi early)
            nc.gpsimd.dma_start(wbf_t[:, c, :], wbf_s[:, c, :])
            nc.gpsimd.dma_start(wbb_t[:, c, :], wbb_s[:, c, :])
        mask_d = [mf_d, mb_d]

        def load_late_weights_a():  # fp8 hidden chunks (needed from step 1)
            for c in range(4, 8):
                nc.sync.dma_start(w8f_t[:, c, :], w8f_s[:, c, :])
                nc.sync.dma_start(w8b_t[:, c, :], w8b_s[:, c, :])

        def load_late_weights_b():  # MLP weights (needed after short streams)
            for c in range(8):
                nc.scalar.dma_start(w1[:, c, :], w1_s[:, c, :])
            for c in range(4):
                nc.scalar.dma_start(w2[:, c, :], w2_s[:, c, :])

        state = {}

        def emit_step(t, d, j, steps):
            w, wh, mw, hi_n, hi_rz = steps[j]
            n = len(steps)
            first = j == 0
            last = j == n - 1
            so = TS - w
            soh = TS - wh if not first else None  # hidden-proj suffix offset
            a0 = t * TS + so
            bb = 16 * d
            pos = (8 - n + j) if d == 0 else (6 + n - j)
            skey = 2 * t + d

            xtb = xt8 = None
            if hi_n:  # n-gate (and maybe r/z) input proj in bf16
                xtb = xbpool.tile([128, 4, TS], BF16, tag="xb", name="xt")
                nc.sync.dma_start(
                    xtb[:, :, so:],
                    xb_d[pos].rearrange("(c k) s -> k c s", k=128)[:, :, a0:a0 + w])
            if not hi_rz:  # r/z input proj in fp8
                xt8 = x8pool.tile([128, 4, TS], F8, tag="x8", name="xt")
                nc.sync.dma_start(
                    xt8[:, :, so:],
                    x8_d[pos].rearrange("(c k) s -> k c s", k=128)[:, :, a0:a0 + w])
            mt = None
            if mw:
                mt = mpool.tile([128, 64], BF16, tag="m", name="mt")
                nc.gpsimd.dma_start(
                    mt[:, :mw],
                    mask_d[d][8 - (n - j), a0:a0 + mw].partition_broadcast(128))

            if first:
                h_prev = h8_prev = None
            else:
                h_prev, h8_prev, _ = state[skey]
            hb_next = (hfin if last else hbf[skey]).tile(
                [128, 4, TS], BF16, tag="hf" if last else f"h{skey}", name="hb")
            h8_next = None
            if not last:
                h8_next = hf8[skey].tile([128, 4, TS], F8, tag=f"g8{skey}",
                                         name="h8")
                nw = steps[j + 1][0]
                if TS - nw < so:  # zero newly exposed prefix for next step's dd
                    nc.gpsimd.memset(hb_next[:, :, TS - nw:so].bitcast(F32), 0.0)

            rps, zps, xpns, ghns = [], [], [], []
            for i in range(4):
                c0 = i * 128
                r_ps = rzps.tile([128, w], F32, tag="rz", name=f"rps{i}")
                z_ps = rzps.tile([128, w], F32, tag="rz", name=f"zps{i}")
                xpn = xpps.tile([128, w], F32, tag="xp", name=f"xpn{i}")
                rps.append(r_ps); zps.append(z_ps); xpns.append(xpn)
                # r/z input projections
                if hi_rz:
                    ww = wbf[d]
                    for k in range(4):
                        st = k == 0
                        lastk = k == 3 and first
                        nc.tensor.matmul(r_ps[:], ww[:, k, c0:c0 + 128],
                                         xtb[:, k, so:], start=st, stop=lastk)
                        nc.tensor.matmul(z_ps[:], ww[:, k, H + c0:H + c0 + 128],
                                         xtb[:, k, so:], start=st, stop=lastk)
                else:
                    ww = w8[d]
                    for p in range(2):
                        st = p == 0
                        lastk = p == 1 and first
                        ksl = slice(2 * p, 2 * p + 2)
                        nc.tensor.matmul(r_ps[:], ww[:, ksl, c0:c0 + 128],
                                         xt8[:, ksl, so:], start=st, stop=lastk,
                                         perf_mode=DR)
                        nc.tensor.matmul(z_ps[:], ww[:, ksl, H + c0:H + c0 + 128],
                                         xt8[:, ksl, so:], start=st, stop=lastk,
                                         perf_mode=DR)
                # n-gate input projection
                if hi_n:
                    ww = wbf[d]
                    for k in range(4):
                        nc.tensor.matmul(xpn[:], ww[:, k, 2 * H + c0:2 * H + c0 + 128],
                                         xtb[:, k, so:], start=k == 0, stop=k == 3)
                else:
                    ww = w8[d]
                    for p in range(2):
                        ksl = slice(2 * p, 2 * p + 2)
                        nc.tensor.matmul(xpn[:], ww[:, ksl, 2 * H + c0:2 * H + c0 + 128],
                                         xt8[:, ksl, so:], start=p == 0, stop=p == 1,
                                         perf_mode=DR)
                if not first:
                    wwh = w8[d]
                    ghn = ghps.tile([128, wh], F32, tag="gh", name=f"ghn{i}")
                    ghns.append(ghn)
                    for p in range(2):
                        ksl = slice(4 + 2 * p, 4 + 2 * p + 2)
                        nc.tensor.matmul(rps[i][:, soh - so:],
                                         wwh[:, ksl, c0:c0 + 128],
                                         h8_prev[:, 2 * p:2 * p + 2, soh:],
                                         start=False, stop=p == 1, perf_mode=DR)
                        nc.tensor.matmul(zps[i][:, soh - so:],
                                         wwh[:, ksl, H + c0:H + c0 + 128],
                                         h8_prev[:, 2 * p:2 * p + 2, soh:],
                                         start=False, stop=p == 1, perf_mode=DR)
                        nc.tensor.matmul(ghn[:],
                                         wwh[:, ksl, 2 * H + c0:2 * H + c0 + 128],
                                         h8_prev[:, 2 * p:2 * p + 2, soh:],
                                         start=p == 0, stop=p == 1, perf_mode=DR)

            # pass A: r/z activations first — they free the rz PSUM banks the
            # next stream-step's input matmuls are waiting on.
            rz_t = []
            for i in range(4):
                if mw:
                    nc.vector.tensor_add(zps[i][:, :mw], zps[i][:, :mw],
                                         mt[:, :mw])
                r = gpool.tile([128, w], BF16, tag="g", name="r")
                nc.scalar.activation(r[:], rps[i][:], ACT.Sigmoid,
                                     bias=bt[:, bb + i:bb + i + 1],
                                     scale=1.0 / 512)
                z = gpool.tile([128, w], BF16, tag="g", name="z")
                nc.scalar.activation(z[:], zps[i][:], ACT.Sigmoid,
                                     bias=bt[:, bb + 4 + i:bb + 5 + i],
                                     scale=1.0 / 512)
                rz_t.append((r, z))
            for i in range(4):
                r, z = rz_t[i]
                tt = gpool.tile([128, w], BF16, tag="g", name="tt")
                if first:
                    # tt = r * bhh_n  via ACT Copy with per-partition scale
                    nc.scalar.activation(tt[:], r[:], ACT.Copy, bias=0.0,
                                         scale=bt[:, bb + 8 + i:bb + 9 + i])
                else:
                    dd = soh - so
                    if dd:
                        nc.scalar.activation(tt[:, :dd], r[:, :dd], ACT.Copy,
                                             bias=0.0,
                                             scale=bt[:, bb + 8 + i:bb + 9 + i])
                    nc.vector.scalar_tensor_tensor(
                        tt[:, dd:], ghns[i][:], bt[:, bb + 8 + i:bb + 9 + i],
                        r[:, dd:], op0=ALU.add, op1=ALU.mult)
                ss = gpool.tile([128, w], BF16, tag="g", name="ss")
                nc.vector.tensor_add(ss[:], tt[:], xpns[i][:])
                nn = gpool.tile([128, w], BF16, tag="g", name="n")
                nc.scalar.activation(nn[:], ss[:], ACT.Tanh,
                                     bias=bt[:, bb + 12 + i:bb + 13 + i],
                                     scale=1.0 / 512)
                # critical chain (feeds next step's hidden matmuls via h8)
                # stays on vector; the bf16 carry copy goes to gpsimd.
                ho = hb_next[:, i, so:]
                if first:
                    e = gpool.tile([128, w], BF16, tag="g", name="e")
                    nc.vector.tensor_mul(e[:], z[:], nn[:])
                    if h8_next is not None:
                        nc.vector.tensor_sub(h8_next[:, i, so:], nn[:], e[:])
                        nc.gpsimd.tensor_sub(ho, nn[:], e[:])
                    else:
                        nc.vector.tensor_sub(ho, nn[:], e[:])
                else:
                    dd_t = gpool.tile([128, w], BF16, tag="g", name="dd")
                    nc.vector.tensor_sub(dd_t[:], h_prev[:, i, so:], nn[:])
                    e = gpool.tile([128, w], BF16, tag="g", name="e")
                    nc.vector.tensor_mul(e[:], z[:], dd_t[:])
                    if h8_next is not None:
                        nc.vector.tensor_add(h8_next[:, i, so:], nn[:], e[:])
                        nc.gpsimd.tensor_add(ho, nn[:], e[:])
                    else:
                        nc.vector.tensor_add(ho, nn[:], e[:])
            state[skey] = (hb_next, h8_next, w)
            return hb_next

        def emit_mlp(t, hf_t, hb_t):
            hid = []
            for i in range(4):
                ps = xpps.tile([128, TS], F32, tag="xp", name="mps")
                for k in range(8):
                    src = hf_t if k < 4 else hb_t
                    nc.tensor.matmul(ps[:], w1[:, k, i * 128:(i + 1) * 128],
                                     src[:, k % 4, :], start=k == 0, stop=k == 7)
                hr = gpool.tile([128, TS], BF16, tag="g", name="hr")
                nc.scalar.activation(hr[:], ps[:], ACT.Relu,
                                     bias=bt[:, 32 + i:33 + i])
                hid.append(hr)
            for i in range(4):
                ps = xpps.tile([128, TS], F32, tag="xp", name="ops")
                for k in range(4):
                    nc.tensor.matmul(ps[:], w2[:, k, i * 128:(i + 1) * 128],
                                     hid[k][:], start=k == 0, stop=k == 3)
                o32 = opool.tile([128, TS], F32, tag="o", name="o32")
                nc.vector.tensor_scalar_add(o32[:], ps[:], bt[:, 36 + i:37 + i])
                nc.sync.dma_start(y_d[i * 128:(i + 1) * 128, t * TS:(t + 1) * TS],
                                  o32[:])

        # End-staggered interleave: later (longer) quarters end later, so
        # every super-step has several streams in flight and the quarter
        # MLPs drain progressively instead of piling up at the end.
        starts = {}
        nmax = 0
        for t in range(NTILES):
            for d in range(2):
                n = len(sched[t][d])
                endoff = NTILES - 1 - t
                starts[(t, d)] = endoff  # provisional: start offset set below
                nmax = max(nmax, n + endoff)
        for t in range(NTILES):
            for d in range(2):
                n = len(sched[t][d])
                endoff = NTILES - 1 - t
                starts[(t, d)] = nmax - n - endoff

        hfs = {}
        mlp_done = set()
        for J in range(nmax):
            for t in range(NTILES):
                for d in range(2):
                    steps = sched[t][d]
                    j = J - starts[(t, d)]
                    if 0 <= j < len(steps):
                        h = emit_step(t, d, j, steps)
                        if j == len(steps) - 1:
                            hfs[(t, d)] = h
            if J == 0:
                load_late_weights_a()
            elif J == 2:
                load_late_weights_b()
            for t in range(NTILES):
                if t not in mlp_done and (t, 0) in hfs and (t, 1) in hfs:
                    emit_mlp(t, hfs[(t, 0)], hfs[(t, 1)])
                    mlp_done.add(t)

    nc.compile()
    return nc


def _mk_sched(lens_pc, t):
    """lens_pc: [BC, NCORES] per-core sorted lengths; tile t rows."""
    seg = lens_pc[t * TS:(t + 1) * TS]  # [TS, NCORES]
    n = int(seg.max())
    steps = []
    for j in range(n):
        need = n - j
        cnt = (seg >= need).sum(axis=0)
        w = min(TS, -(-int(cnt.max()) // 16) * 16)
        mw = int(w - int(cnt.min()))
        hi_n = j >= n - N_HI_N
        hi_rz = j >= n - N_HI_RZ
        steps.append([w, 0, mw, hi_n, hi_rz])
    for j in range(1, n):
        steps[j][1] = steps[j - 1][0]  # hidden width = prev step width
    return tuple(tuple(s) for s in steps)


def kernel(padded_window, window_len, Wih_f, Whh_f, bih_f, bhh_f,
           Wih_b, Whh_b, bih_b, bhh_b, W1, b1, W2, b2):
    wl = np.asarray(window_len)
    lf = (wl - 1) // 2 + 1
    lb = wl // 2 + 1
    order = np.argsort(wl, kind="stable")

    lf_pc = lf[order].reshape(-1, NCORES)
    lb_pc = lb[order].reshape(-1, NCORES)

    sched = tuple((_mk_sched(lf_pc, t), _mk_sched(lb_pc, t))
                  for t in range(NTILES))

    if sched not in _PROGRAM_CACHE:
        _PROGRAM_CACHE[sched] = _build_program(sched)
    nc = _PROGRAM_CACHE[sched]

    f32 = np.float32
    wf_full = np.concatenate([Wih_f.T, Whh_f.T], 0).astype(f32) * 512.0
    wb_full = np.concatenate([Wih_b.T, Whh_b.T], 0).astype(f32) * 512.0
    w8f = np.clip(wf_full, -240, 240).astype(NP_F8)
    w8b = np.clip(wb_full, -240, 240).astype(NP_F8)
    wbf = wf_full[:D].astype(NP_BF)
    wbb = wb_full[:D].astype(NP_BF)
    w1 = np.ascontiguousarray(W1.T, dtype=f32).astype(NP_BF)
    w2 = np.ascontiguousarray(W2.T, dtype=f32).astype(NP_BF)

    def chunks(v):
        return np.asarray(v, f32).reshape(4, 128)

    bias = np.concatenate([
        chunks((bih_f + bhh_f)[:H]), chunks((bih_f + bhh_f)[H:2 * H]),
        chunks(bhh_f[2 * H:] * 512.0), chunks(bih_f[2 * H:]),
        chunks((bih_b + bhh_b)[:H]), chunks((bih_b + bhh_b)[H:2 * H]),
        chunks(bhh_b[2 * H:] * 512.0), chunks(bih_b[2 * H:]),
        chunks(b1), chunks(b2),
    ], 0)  # [40, 128]

    pw = np.asarray(padded_window, f32)
    in_maps = []
    p8 = np.arange(8)
    for c in range(NCORES):
        idx = order[c::NCORES]
        xT = np.ascontiguousarray(pw[idx].transpose(1, 2, 0))  # [15, 512, BC]
        mzf = (512.0 * BIG * (p8[:, None] < (8 - lf[idx])[None, :])).astype(NP_BF)
        mzb = (512.0 * BIG * (p8[:, None] < (8 - lb[idx])[None, :])).astype(NP_BF)
        in_maps.append({
            "x8": np.clip(xT, -240, 240).astype(NP_F8),
            "xb": xT.astype(NP_BF),
            "w8f": w8f, "w8b": w8b, "wbf": wbf, "wbb": wbb,
            "w1": w1, "w2": w2, "bias": bias,
            "maskzf": mzf, "maskzb": mzb,
        })

    trace = bool(os.environ.get("GRU_TRACE"))
    kw = {}
    if os.environ.get("GRU_TMPDIR"):
        kw["tmpdir"] = os.environ["GRU_TMPDIR"]
    res = run_bass_kernel_spmd(nc, in_maps, core_ids=list(range(NCORES)),
                               trace=trace, **kw)
    global LAST_RESULT
    LAST_RESULT = res
    out = np.empty((B, H), f32)
    for c in range(NCORES):
        out[order[c::NCORES]] = res.results[c]["y"].T
    return out


# revision 19
# speedup vs baseline: 1.2316x; 1.2217x over previous
"""BiGRU encoder kernel for 8 Trainium2 NeuronCores.

Strategy (v3, fp8 DoubleRow + quarter-tile streams):
  - Masked GRU over FIXED position ranges: forward runs positions (8-n)..7
    ascending, backward (6+n)..7 descending; a sample of length l starts at
    step n-l with h=0 (prefix memset) and a +BIG z-gate mask keeps
    over-included samples at exactly h=0 until their true start.
  - Sort samples by window_len, deal round-robin to 8 cores; per core FOUR
    batch tiles of 256 sorted samples -> 8 independent (tile, direction)
    streams.  Streams are end-staggered so every phase of the kernel has
    several streams in flight (gate latency of one hides under matmuls of
    others).  Step widths are EXACT per-step active counts (max over cores,
    rounded to 16 for alignment); the <=16 over-included samples are fixed
    by a narrow z-mask add into PSUM.
  - Matmuls: fp8e4 DoubleRow (K=256 per instruction, measured 2x throughput)
    for ALL hidden projections and for input projections except the last
    N_HI=3 steps of each stream, which run in bf16 for accuracy.  Weights are
    pre-scaled by 512 (exact power of 2) so unscaled fp8 x/h stay in e4m3's
    normal range; activations rescale with scale=1/512.
  - h is carried in bf16 (fp8 carry compounds error); the fp8 copy for the
    next step's matmul is written by vector (critical path), the bf16 carry
    by gpsimd (off critical path).
  - Hidden projections run at the PREVIOUS step's width; the n-gate
    pre-activation prefix (newly started samples) is r*bhh_n via a scalar
    ACT Copy with per-partition scale.
  - Output is written feature-major [H, Bc]; the host transposes (free).
"""

import os
from contextlib import ExitStack

import numpy as np
import ml_dtypes

import concourse.bacc as bacc
import concourse.tile as tile
from concourse import mybir
from concourse.bass_utils import run_bass_kernel_spmd

NCORES = 8
B, T, D, H = 8192, 15, 512, 512
G = 3 * H
BIG = 40.0
TS = 512             # samples per batch tile
NTILES = 2
BC = TS * NTILES     # samples per core
N_HI_N = int(os.environ.get("GRU_NHI_N", "4"))   # n-gate bf16 on last-k steps
N_HI_RZ = int(os.environ.get("GRU_NHI_RZ", "1"))  # r/z bf16 on last-k steps
F32 = mybir.dt.float32
BF16 = mybir.dt.bfloat16
F8 = mybir.dt.float8e4
DR = mybir.MatmulPerfMode.DoubleRow

ACT = mybir.ActivationFunctionType
ALU = mybir.AluOpType

NP_BF = ml_dtypes.bfloat16
NP_F8 = ml_dtypes.float8_e4m3

_PROGRAM_CACHE = {}
LAST_RESULT = None


def _build_program(sched):
    """sched[t][d] = tuple of (w, w_prev_hidden, mw, hi) per step."""
    nc = bacc.Bacc("TRN2", target_bir_lowering=False, debug=False,
                   num_devices=NCORES)

    x8_d = nc.dram_tensor("x8", [T, D, BC], F8, kind="ExternalInput")
    xb_d = nc.dram_tensor("xb", [T, D, BC], BF16, kind="ExternalInput")
    w8f_d = nc.dram_tensor("w8f", [D + H, G], F8, kind="ExternalInput")
    w8b_d = nc.dram_tensor("w8b", [D + H, G], F8, kind="ExternalInput")
    wbf_d = nc.dram_tensor("wbf", [D, G], BF16, kind="ExternalInput")
    wbb_d = nc.dram_tensor("wbb", [D, G], BF16, kind="ExternalInput")
    w1_d = nc.dram_tensor("w1", [2 * H, H], BF16, kind="ExternalInput")
    w2_d = nc.dram_tensor("w2", [H, H], BF16, kind="ExternalInput")
    bias_d = nc.dram_tensor("bias", [40, 128], F32, kind="ExternalInput")
    mf_d = nc.dram_tensor("maskzf", [8, BC], BF16, kind="ExternalInput")
    mb_d = nc.dram_tensor("maskzb", [8, BC], BF16, kind="ExternalInput")
    y_d = nc.dram_tensor("y", [H, BC], F32, kind="ExternalOutput")

    NS = 2 * NTILES  # streams
    with tile.TileContext(nc) as tc, ExitStack() as ctx:
        const = ctx.enter_context(tc.tile_pool(name="const", bufs=1))
        x8pool = ctx.enter_context(tc.tile_pool(name="x8", bufs=6))
        xbpool = ctx.enter_context(tc.tile_pool(name="xb", bufs=6))
        hbf = [ctx.enter_context(tc.tile_pool(name=f"hb{s}", bufs=2))
               for s in range(NS)]
        hf8 = [ctx.enter_context(tc.tile_pool(name=f"h8{s}", bufs=2))
               for s in range(NS)]
        hfin = ctx.enter_context(tc.tile_pool(name="hfin", bufs=NS))
        gpool = ctx.enter_context(tc.tile_pool(name="g", bufs=28))
        mpool = ctx.enter_context(tc.tile_pool(name="m", bufs=2))
        opool = ctx.enter_context(tc.tile_pool(name="o", bufs=2))
        rzps = ctx.enter_context(tc.tile_pool(name="rz", bufs=5, space="PSUM"))
        xpps = ctx.enter_context(tc.tile_pool(name="xp", bufs=2, space="PSUM"))
        ghps = ctx.enter_context(tc.tile_pool(name="gh", bufs=1, space="PSUM"))

        def wtile(dram, kchunks, cols, dt, name):
            t_ = const.tile([128, kchunks, cols], dt, name=name)
            return t_, dram.rearrange("(c k) g -> k c g", k=128)

        w8f_t, w8f_s = wtile(w8f_d, 8, G, F8, "w8f")
        w8b_t, w8b_s = wtile(w8b_d, 8, G, F8, "w8b")
        wbf_t, wbf_s = wtile(wbf_d, 4, G, BF16, "wbf")
        wbb_t, wbb_s = wtile(wbb_d, 4, G, BF16, "wbb")
        w1, w1_s = wtile(w1_d, 8, H, BF16, "w1")
        w2, w2_s = wtile(w2_d, 4, H, BF16, "w2")
        w8 = [w8f_t, w8b_t]
        wbf = [wbf_t, wbb_t]
        bt = const.tile([128, 40], F32)
        nc.gpsimd.dma_start(bt[:], bias_d.rearrange("n p -> p n"))
        # DMA order = order of first use.  The scalar queue must stay clear
        # early (ACT table load + first gates run there); weights go on
        # sync/gpsimd in need-order, w1/w2 go on scalar after step 2.
        for c in range(4):  # fp8 input chunks first (step 0 needs them)
            nc.sync.dma_start(w8f_t[:, c, :], w8f_s[:, c, :])
            nc.gpsimd.dma_start(w8b_t[:, c, :], w8b_s[:, c, :])
        for c in range(4):  # bf16 input weights (short streams hit hi early)
            nc.gpsimd.dma_start(wbf_t[:, c, :], wbf_s[:, c, :])
            nc.gpsimd.dma_start(wbb_t[:, c, :], wbb_s[:, c, :])
        mask_d = [mf_d, mb_d]

        def load_late_weights_a():  # fp8 hidden chunks (needed from step 1)
            for c in range(4, 8):
                nc.sync.dma_start(w8f_t[:, c, :], w8f_s[:, c, :])
                nc.sync.dma_start(w8b_t[:, c, :], w8b_s[:, c, :])

        def load_late_weights_b():  # MLP weights (needed after short streams)
            for c in range(8):
                nc.scalar.dma_start(w1[:, c, :], w1_s[:, c, :])
            for c in range(4):
                nc.scalar.dma_start(w2[:, c, :], w2_s[:, c, :])

        state = {}

        def emit_step(t, d, j, steps):
            w, wh, mw, hi_n, hi_rz = steps[j]
            n = len(steps)
            first = j == 0
            last = j == n - 1
            so = TS - w
            soh = TS - wh if not first else None  # hidden-proj suffix offset
            a0 = t * TS + so
            bb = 16 * d
            pos = (8 - n + j) if d == 0 else (6 + n - j)
            skey = 2 * t + d

            xtb = xt8 = None
            if hi_n:  # n-gate (and maybe r/z) input proj in bf16
                xtb = xbpool.tile([128, 4, TS], BF16, tag="xb", name="xt")
                nc.sync.dma_start(
                    xtb[:, :, so:],
                    xb_d[pos].rearrange("(c k) s -> k c s", k=128)[:, :, a0:a0 + w])
            if not hi_rz:  # r/z input proj in fp8
                xt8 = x8pool.tile([128, 4, TS], F8, tag="x8", name="xt")
                nc.sync.dma_start(
                    xt8[:, :, so:],
                    x8_d[pos].rearrange("(c k) s -> k c s", k=128)[:, :, a0:a0 + w])
            mt = None
            if mw:
                mt = mpool.tile([128, 64], BF16, tag="m", name="mt")
                nc.gpsimd.dma_start(
                    mt[:, :mw],
                    mask_d[d][8 - (n - j), a0:a0 + mw].partition_broadcast(128))

            if first:
                h_prev = h8_prev = None
            else:
                h_prev, h8_prev, _ = state[skey]
            hb_next = (hfin if last else hbf[skey]).tile(
                [128, 4, TS], BF16, tag="hf" if last else f"h{skey}", name="hb")
            h8_next = None
            if not last:
                h8_next = hf8[skey].tile([128, 4, TS], F8, tag=f"g8{skey}",
                                         name="h8")
                nw = steps[j + 1][0]
                if TS - nw < so:  # zero newly exposed prefix for next step's dd
                    nc.gpsimd.memset(hb_next[:, :, TS - nw:so].bitcast(F32), 0.0)

            rps, zps, xpns, ghns = [], [], [], []
            for i in range(4):
                c0 = i * 128
                r_ps = rzps.tile([128, w], F32, tag="rz", name=f"rps{i}")
                z_ps = rzps.tile([128, w], F32, tag="rz", name=f"zps{i}")
                xpn = xpps.tile([128, w], F32, tag="xp", name=f"xpn{i}")
                rps.append(r_ps); zps.append(z_ps); xpns.append(xpn)
                # r/z input projections
                if hi_rz:
                    ww = wbf[d]
                    for k in range(4):
                        st = k == 0
                        lastk = k == 3 and first
                        nc.tensor.matmul(r_ps[:], ww[:, k, c0:c0 + 128],
                                         xtb[:, k, so:], start=st, stop=lastk)
                        nc.tensor.matmul(z_ps[:], ww[:, k, H + c0:H + c0 + 128],
                                         xtb[:, k, so:], start=st, stop=lastk)
                else:
                    ww = w8[d]
                    for p in range(2):
                        st = p == 0
                        lastk = p == 1 and first
                        ksl = slice(2 * p, 2 * p + 2)
                        nc.tensor.matmul(r_ps[:], ww[:, ksl, c0:c0 + 128],
                                         xt8[:, ksl, so:], start=st, stop=lastk,
                                         perf_mode=DR)
                        nc.tensor.matmul(z_ps[:], ww[:, ksl, H + c0:H + c0 + 128],
                                         xt8[:, ksl, so:], start=st, stop=lastk,
                                         perf_mode=DR)
                # n-gate input projection
                if hi_n:
                    ww = wbf[d]
                    for k in range(4):
                        nc.tensor.matmul(xpn[:], ww[:, k, 2 * H + c0:2 * H + c0 + 128],
                                         xtb[:, k, so:], start=k == 0, stop=k == 3)
                else:
                    ww = w8[d]
                    for p in range(2):
                        ksl = slice(2 * p, 2 * p + 2)
                        nc.tensor.matmul(xpn[:], ww[:, ksl, 2 * H + c0:2 * H + c0 + 128],
                                         xt8[:, ksl, so:], start=p == 0, stop=p == 1,
                                         perf_mode=DR)
                if not first:
                    wwh = w8[d]
                    ghn = ghps.tile([128, wh], F32, tag="gh", name=f"ghn{i}")
                    ghns.append(ghn)
                    for p in range(2):
                        ksl = slice(4 + 2 * p, 4 + 2 * p + 2)
                        nc.tensor.matmul(rps[i][:, soh - so:],
                                         wwh[:, ksl, c0:c0 + 128],
                                         h8_prev[:, 2 * p:2 * p + 2, soh:],
                                         start=False, stop=p == 1, perf_mode=DR)
                        nc.tensor.matmul(zps[i][:, soh - so:],
                                         wwh[:, ksl, H + c0:H + c0 + 128],
                                         h8_prev[:, 2 * p:2 * p + 2, soh:],
                                         start=False, stop=p == 1, perf_mode=DR)
                        nc.tensor.matmul(ghn[:],
                                         wwh[:, ksl, 2 * H + c0:2 * H + c0 + 128],
                                         h8_prev[:, 2 * p:2 * p + 2, soh:],
                                         start=p == 0, stop=p == 1, perf_mode=DR)

            for i in range(4):
                if mw:
                    nc.vector.tensor_add(zps[i][:, :mw], zps[i][:, :mw],
                                         mt[:, :mw])
                r = gpool.tile([128, w], BF16, tag="g", name="r")
                nc.scalar.activation(r[:], rps[i][:], ACT.Sigmoid,
                                     bias=bt[:, bb + i:bb + i + 1],
                                     scale=1.0 / 512)
                z = gpool.tile([128, w], BF16, tag="g", name="z")
                nc.scalar.activation(z[:], zps[i][:], ACT.Sigmoid,
                                     bias=bt[:, bb + 4 + i:bb + 5 + i],
                                     scale=1.0 / 512)
                tt = gpool.tile([128, w], BF16, tag="g", name="tt")
                if first:
                    # tt = r * bhh_n  via ACT Copy with per-partition scale
                    nc.scalar.activation(tt[:], r[:], ACT.Copy, bias=0.0,
                                         scale=bt[:, bb + 8 + i:bb + 9 + i])
                else:
                    dd = soh - so
                    if dd:
                        nc.scalar.activation(tt[:, :dd], r[:, :dd], ACT.Copy,
                                             bias=0.0,
                                             scale=bt[:, bb + 8 + i:bb + 9 + i])
                    nc.vector.scalar_tensor_tensor(
                        tt[:, dd:], ghns[i][:], bt[:, bb + 8 + i:bb + 9 + i],
                        r[:, dd:], op0=ALU.add, op1=ALU.mult)
                ss = gpool.tile([128, w], BF16, tag="g", name="ss")
                nc.vector.tensor_add(ss[:], tt[:], xpns[i][:])
                nn = gpool.tile([128, w], BF16, tag="g", name="n")
                nc.scalar.activation(nn[:], ss[:], ACT.Tanh,
                                     bias=bt[:, bb + 12 + i:bb + 13 + i],
                                     scale=1.0 / 512)
                # critical chain (feeds next step's hidden matmuls via h8)
                # stays on vector; the bf16 carry copy goes to gpsimd.
                ho = hb_next[:, i, so:]
                if first:
                    e = gpool.tile([128, w], BF16, tag="g", name="e")
                    nc.vector.tensor_mul(e[:], z[:], nn[:])
                    if h8_next is not None:
                        nc.vector.tensor_sub(h8_next[:, i, so:], nn[:], e[:])
                        nc.gpsimd.tensor_sub(ho, nn[:], e[:])
                    else:
                        nc.vector.tensor_sub(ho, nn[:], e[:])
                else:
                    dd_t = gpool.tile([128, w], BF16, tag="g", name="dd")
                    nc.vector.tensor_sub(dd_t[:], h_prev[:, i, so:], nn[:])
                    e = gpool.tile([128, w], BF16, tag="g", name="e")
                    nc.vector.tensor_mul(e[:], z[:], dd_t[:])
                    if h8_next is not None:
                        nc.vector.tensor_add(h8_next[:, i, so:], nn[:], e[:])
                        nc.gpsimd.tensor_add(ho, nn[:], e[:])
                    else:
                        nc.vector.tensor_add(ho, nn[:], e[:])
            state[skey] = (hb_next, h8_next, w)
            return hb_next

        def emit_mlp(t, hf_t, hb_t):
            hid = []
            for i in range(4):
                ps = xpps.tile([128, TS], F32, tag="xp", name="mps")
                for k in range(8):
                    src = hf_t if k < 4 else hb_t
                    nc.tensor.matmul(ps[:], w1[:, k, i * 128:(i + 1) * 128],
                                     src[:, k % 4, :], start=k == 0, stop=k == 7)
                hr = gpool.tile([128, TS], BF16, tag="g", name="hr")
                nc.scalar.activation(hr[:], ps[:], ACT.Relu,
                                     bias=bt[:, 32 + i:33 + i])
                hid.append(hr)
            for i in range(4):
                ps = xpps.tile([128, TS], F32, tag="xp", name="ops")
                for k in range(4):
                    nc.tensor.matmul(ps[:], w2[:, k, i * 128:(i + 1) * 128],
                                     hid[k][:], start=k == 0, stop=k == 3)
                o32 = opool.tile([128, TS], F32, tag="o", name="o32")
                nc.vector.tensor_scalar_add(o32[:], ps[:], bt[:, 36 + i:37 + i])
                nc.sync.dma_start(y_d[i * 128:(i + 1) * 128, t * TS:(t + 1) * TS],
                                  o32[:])

        # End-staggered interleave: later (longer) quarters end later, so
        # every super-step has several streams in flight and the quarter
        # MLPs drain progressively instead of piling up at the end.
        starts = {}
        nmax = 0
        for t in range(NTILES):
            for d in range(2):
                n = len(sched[t][d])
                endoff = NTILES - 1 - t
                starts[(t, d)] = endoff  # provisional: start offset set below
                nmax = max(nmax, n + endoff)
        for t in range(NTILES):
            for d in range(2):
                n = len(sched[t][d])
                endoff = NTILES - 1 - t
                starts[(t, d)] = nmax - n - endoff

        hfs = {}
        mlp_done = set()
        for J in range(nmax):
            for t in range(NTILES):
                for d in range(2):
                    steps = sched[t][d]
                    j = J - starts[(t, d)]
                    if 0 <= j < len(steps):
                        h = emit_step(t, d, j, steps)
                        if j == len(steps) - 1:
                            hfs[(t, d)] = h
            if J == 0:
                load_late_weights_a()
            elif J == 2:
                load_late_weights_b()
            for t in range(NTILES):
                if t not in mlp_done and (t, 0) in hfs and (t, 1) in hfs:
                    emit_mlp(t, hfs[(t, 0)], hfs[(t, 1)])
                    mlp_done.add(t)

    nc.compile()
    return nc


def _mk_sched(lens_pc, t):
    """lens_pc: [BC, NCORES] per-core sorted lengths; tile t rows."""
    seg = lens_pc[t * TS:(t + 1) * TS]  # [TS, NCORES]
    n = int(seg.max())
    steps = []
    for j in range(n):
        need = n - j
        cnt = (seg >= need).sum(axis=0)
        w = min(TS, -(-int(cnt.max()) // 16) * 16)
        mw = int(w - int(cnt.min()))
        hi_n = j >= n - N_HI_N
        hi_rz = j >= n - N_HI_RZ
        steps.append([w, 0, mw, hi_n, hi_rz])
    for j in range(1, n):
        steps[j][1] = steps[j - 1][0]  # hidden width = prev step width
    return tuple(tuple(s) for s in steps)


def kernel(padded_window, window_len, Wih_f, Whh_f, bih_f, bhh_f,
           Wih_b, Whh_b, bih_b, bhh_b, W1, b1, W2, b2):
    wl = np.asarray(window_len)
    lf = (wl - 1) // 2 + 1
    lb = wl // 2 + 1
    order = np.argsort(wl, kind="stable")

    lf_pc = lf[order].reshape(-1, NCORES)
    lb_pc = lb[order].reshape(-1, NCORES)

    sched = tuple((_mk_sched(lf_pc, t), _mk_sched(lb_pc, t))
                  for t in range(NTILES))

    if sched not in _PROGRAM_CACHE:
        _PROGRAM_CACHE[sched] = _build_program(sched)
    nc = _PROGRAM_CACHE[sched]

    f32 = np.float32
    wf_full = np.concatenate([Wih_f.T, Whh_f.T], 0).astype(f32) * 512.0
    wb_full = np.concatenate([Wih_b.T, Whh_b.T], 0).astype(f32) * 512.0
    w8f = np.clip(wf_full, -240, 240).astype(NP_F8)
    w8b = np.clip(wb_full, -240, 240).astype(NP_F8)
    wbf = wf_full[:D].astype(NP_BF)
    wbb = wb_full[:D].astype(NP_BF)
    w1 = np.ascontiguousarray(W1.T, dtype=f32).astype(NP_BF)
    w2 = np.ascontiguousarray(W2.T, dtype=f32).astype(NP_BF)

    def chunks(v):
        return np.asarray(v, f32).reshape(4, 128)

    bias = np.concatenate([
        chunks((bih_f + bhh_f)[:H]), chunks((bih_f + bhh_f)[H:2 * H]),
        chunks(bhh_f[2 * H:] * 512.0), chunks(bih_f[2 * H:]),
        chunks((bih_b + bhh_b)[:H]), chunks((bih_b + bhh_b)[H:2 * H]),
        chunks(bhh_b[2 * H:] * 512.0), chunks(bih_b[2 * H:]),
        chunks(b1), chunks(b2),
    ], 0)  # [40, 128]

    pw = np.asarray(padded_window, f32)
    in_maps = []
    p8 = np.arange(8)
    for c in range(NCORES):
        idx = order[c::NCORES]
        xT = np.ascontiguousarray(pw[idx].transpose(1, 2, 0))  # [15, 512, BC]
        mzf = (512.0 * BIG * (p8[:, None] < (8 - lf[idx])[None, :])).astype(NP_BF)
        mzb = (512.0 * BIG * (p8[:, None] < (8 - lb[idx])[None, :])).astype(NP_BF)
        in_maps.append({
            "x8": np.clip(xT, -240, 240).astype(NP_F8),
            "xb": xT.astype(NP_BF),
            "w8f": w8f, "w8b": w8b, "wbf": wbf, "wbb": wbb,
            "w1": w1, "w2": w2, "bias": bias,
            "maskzf": mzf, "maskzb": mzb,
        })

    trace = bool(os.environ.get("GRU_TRACE"))
    kw = {}
    if os.environ.get("GRU_TMPDIR"):
        kw["tmpdir"] = os.environ["GRU_TMPDIR"]
    res = run_bass_kernel_spmd(nc, in_maps, core_ids=list(range(NCORES)),
                               trace=trace, **kw)
    global LAST_RESULT
    LAST_RESULT = res
    out = np.empty((B, H), f32)
    for c in range(NCORES):
        out[order[c::NCORES]] = res.results[c]["y"].T
    return out
